# revision 29
# baseline (speedup 1.0000x reference)
"""Trainium2 Bass kernel for nn_Encoder_55362128445616.

Transformer encoder layer: B=8, S=1024, D=512, single-head attention over
H*D=4096. Sharding: data-parallel over batch, one batch element per core,
no collectives.

Key algebraic folding (host-side, exact):
  scores = Q K^T / s = x (Wq^T Wk / s) x^T  (+ per-k bias from bq; per-q
  terms cancel in softmax), so the 4096-dim QKV contractions collapse to
  512-dim ones via M = Wq^T Wk / s and NT = (Wo Wv)^T.  LN1's affine is
  folded into W1/b1; all matmuls run as fp32r (1 cycle/row on PE).

Two compiled variants: `fast` (biases zero, LN affines identity — matches
the reference's setup_inputs) and a general fallback.
"""

import math

import numpy as np

# If the environment sets BASS_TRACE, bass_utils imports antenv.axon_hooks,
# which this image may lack — provide a no-op stub so plain runs never crash.
import sys as _sys
import types as _types
try:
    import antenv.axon_hooks  # noqa: F401
except ImportError:
    _m = _types.ModuleType("antenv.axon_hooks")
    _m.get_axon_ntff_profile_hook = lambda: None
    _m.set_axon_ntff_profile_hook = lambda hook: None
    _sys.modules["antenv.axon_hooks"] = _m

import concourse.bacc as bacc
import concourse.mybir as mybir
import concourse.tile as tile
from concourse.bass_utils import run_bass_kernel_spmd

B, S, D = 8, 1024, 512
NQ = S // 128   # 8 q/k tiles of 128
ND = D // 128   # 4 d tiles of 128
F32 = mybir.dt.float32
F32R = mybir.dt.float32r
AF = mybir.ActivationFunctionType
AX = mybir.AxisListType

_BUILT = {}


def _build(fast):
    if fast in _BUILT:
        return _BUILT[fast]

    nc = bacc.Bacc("TRN2", target_bir_lowering=False, debug=False, num_devices=B)

    def din(name, shape, dt=F32R):
        return nc.dram_tensor(name, shape, dt, kind="ExternalInput").ap()

    # all big inputs are pre-arranged on host to the exact SBUF layout so
    # every load is one fully-contiguous DMA at max HBM rate
    xT_d = din("xT", [2, 128, ND * 512])  # x[b].T as [q-half][p][bt*512]
    xn_d = din("xn", [128, NQ * D])       # x[b] as [p][kt*D]
    M_d = din("Mw", [128, ND * D])        # Wq^T Wk / sqrt(D), [p][bt*D]
    NT_d = din("NT", [128, ND * D])
    W1gT_d = din("W1gT", [128, ND * D])
    W2T_d = din("W2T", [128, ND * D])
    # packed per-partition columns: [0:8]=abias, [8:10]=ones, [10:14]=c1, [14:15]=eps
    sm_d = din("smalls", [128, 16], F32)
    id_d = din("ident", [128, 128], F32)
    if not fast:
        xres_d = din("xres", [S, D], F32)     # x[b] + (Wo@bv + bo)
        C2_d = din("C2", [1, D])              # b2 + be0
        onesr_d = din("onesr", [1, 128])
        g0b_d = din("g0b", [128, D], F32)
        g1b_d = din("g1b", [128, D], F32)
        be1b_d = din("be1b", [128, D], F32)
    out_d = nc.dram_tensor("out", [S, D], F32, kind="ExternalOutput").ap()

    with tile.TileContext(nc) as tc:
        with (
            tc.tile_pool(name="res", bufs=1) as res,
            tc.tile_pool(name="work", bufs=2) as work,
            tc.tile_pool(name="small", bufs=8) as small,
            tc.tile_pool(name="psA", bufs=3, space="PSUM") as psA,
            tc.tile_pool(name="psS", bufs=2, space="PSUM") as psS,
            tc.tile_pool(name="psD", bufs=1, space="PSUM") as psD,
        ):
            # ---- resident loads: sync (HWDGE) carries the phase-1/2 critical
            # path in use-order; gpsimd (SWDGE) carries later-phase tensors. ----
            Mw = res.tile([128, ND, D], F32R)
            nc.sync.dma_start(Mw[:], M_d.rearrange("p (t n) -> p t n", n=D))
            xT = res.tile([128, ND, S], F32R, tag="zx")
            for h in range(2):
                nc.sync.dma_start(
                    xT[:, :, h * 512:(h + 1) * 512],
                    xT_d[h].rearrange("p (t q) -> p t q", q=512))
            sm = res.tile([128, 16], F32)
            nc.sync.dma_start(sm[:], sm_d[:])
            xn = res.tile([128, NQ, D], F32R)
            nc.sync.dma_start(xn[:], xn_d.rearrange("p (t n) -> p t n", n=D))

            ident = res.tile([128, 128], F32)
            nc.sync.dma_start(ident[:], id_d[:])
            NTw = res.tile([128, ND, D], F32R)
            nc.sync.dma_start(NTw[:], NT_d.rearrange("p (t n) -> p t n", n=D))
            W1gT = res.tile([128, ND, D], F32R)
            nc.sync.dma_start(W1gT[:], W1gT_d.rearrange("p (t n) -> p t n", n=D))
            W2T = res.tile([128, ND, D], F32R)
            nc.sync.dma_start(W2T[:], W2T_d.rearrange("p (t n) -> p t n", n=D))
            if not fast:
                xres = res.tile([128, NQ, D], F32)
                nc.sync.dma_start(xres[:], xres_d.rearrange("(t p) n -> p t n", p=128))
                C2 = res.tile([1, D], F32R)
                nc.sync.dma_start(C2[:], C2_d[:])
                onesr = res.tile([1, 128], F32R)
                nc.sync.dma_start(onesr[:], onesr_d[:])
                g0b = res.tile([128, D], F32)
                nc.sync.dma_start(g0b[:], g0b_d[:])
                g1b = res.tile([128, D], F32)
                nc.sync.dma_start(g1b[:], g1b_d[:])
                be1b = res.tile([128, D], F32)
                nc.sync.dma_start(be1b[:], be1b_d[:])

            abias = sm[:, 0:8]
            onesc = sm[:, 8:10]
            c1 = sm[:, 10:14]
            epsT = sm[:, 14:15]

            # HAM warm-up on an uninitialized scratch tile: no input deps, so
            # the PE starts immediately while the first DMAs land.
            wtile = res.tile([128, 128], F32)
            nc.vector.memset(wtile[:], 0.0)
            for w in range(20):
                psw = psA.tile([128, 128], F32, tag="a", name=f"psw{w}")
                nc.tensor.matmul(psw[:], wtile[:], wtile[:], start=True, stop=True)

            # big SBUF intermediates; "mid"/"zx" tags reuse slots
            IN2 = res.tile([128, ND, S], F32R, tag="mid")   # M^T x^T
            PT = res.tile([128, NQ, S], F32R)               # exp(scores^T)
            zT = res.tile([128, ND, S], F32R)
            ff1T = res.tile([128, ND, S], F32R)

            # ---- phase 1: IN2[a, q] = sum_b M[b, a] xT[b, q] ----
            for qc in range(2):
                for at in range(ND):
                    ps = psA.tile([128, 512], F32, tag="a")
                    for bt in range(ND):
                        nc.tensor.matmul(
                            ps[:],
                            Mw[:, bt, at * 128:(at + 1) * 128],
                            xT[:, bt, qc * 512:(qc + 1) * 512],
                            start=(bt == 0), stop=(bt == ND - 1),
                        )
                    nc.vector.tensor_copy(IN2[:, at, qc * 512:(qc + 1) * 512], ps[:])

            # ---- phase 2: scoresT[k, q] = sum_a x[k, a] IN2[a, q]; PT = exp ----
            for kt in range(NQ):
                ps = psS.tile([128, 1024], F32, tag="s")
                for qc in range(2):
                    for at in range(ND):
                        nc.tensor.matmul(
                            ps[:, qc * 512:(qc + 1) * 512],
                            xT[:, at, kt * 128:(kt + 1) * 128],
                            IN2[:, at, qc * 512:(qc + 1) * 512],
                            start=(at == 0), stop=(at == ND - 1),
                        )
                bias = 0.0 if fast else abias[:, kt:kt + 1]
                nc.scalar.activation(PT[:, kt, :], ps[:], AF.Exp, bias=bias)

            # denomT[q] = sum_k PT[k, q]: DVE tree-sums the 8 k-tiles, then one
            # cheap f32 ones-matmul per q-tile flips [k-part, q] to [q-part, 1].
            dps = psD.tile([128, 2 * NQ], F32, tag="d")
            for qc in range(2):
                qs = slice(qc * 512, (qc + 1) * 512)
                dacc = work.tile([128, 512], F32, tag="sq", name=f"dacc{qc}")
                nc.vector.tensor_add(dacc[:], PT[:, 0, qs].bitcast(F32), PT[:, 1, qs].bitcast(F32))
                for kt in range(2, NQ):
                    nc.vector.tensor_add(dacc[:], dacc[:], PT[:, kt, qs].bitcast(F32))
                for ql in range(4):
                    qt = qc * 4 + ql
                    nc.tensor.matmul(
                        dps[:, 2 * qt:2 * qt + 2],
                        dacc[:, ql * 128:(ql + 1) * 128],
                        onesc[:],
                        start=True, stop=True,
                    )
            recip = res.tile([128, 2 * NQ], F32)
            nc.vector.reciprocal(recip[:], dps[:])

            # ---- phases 3-6, interleaved so PE work (ST qc1, ff1 chunks)
            # fills the LN1-chain latency windows ----
            ST = res.tile([128, ND, S], F32R, tag="mid")
            z = res.tile([128, NQ, D], F32, tag="zx")

            def st_chunk(qc):
                for dt in range(ND):
                    ps = psA.tile([128, 512], F32, tag="a", name="ps_st")
                    for kt in range(NQ):
                        nc.tensor.matmul(
                            ps[:],
                            xn[:, kt, dt * 128:(dt + 1) * 128],
                            PT[:, kt, qc * 512:(qc + 1) * 512],
                            start=(kt == 0), stop=(kt == NQ - 1),
                        )
                    nc.scalar.copy(ST[:, dt, qc * 512:(qc + 1) * 512], ps[:])

            def mha_stats(qts):
                x1s, lns = [], []
                for qt in qts:
                    ps = psA.tile([128, 512], F32, tag="a", name="ps_mha")
                    for dt in range(ND):
                        nc.tensor.matmul(
                            ps[:],
                            ST[:, dt, qt * 128:(qt + 1) * 128],
                            NTw[:, dt, :],
                            start=(dt == 0), stop=(dt == ND - 1),
                        )
                    x1 = work.tile([128, D], F32, tag="x1",
                                   bufs=NQ if fast else 4, name=f"x1_{qt}")
                    s1 = small.tile([128, 1], F32, tag="s1")
                    resid = xn[:, qt, :].bitcast(F32) if fast else xres[:, qt, :]
                    nc.vector.scalar_tensor_tensor(
                        x1[:], ps[:], recip[:, 2 * qt:2 * qt + 1], resid,
                        op0=mybir.AluOpType.mult, op1=mybir.AluOpType.add,
                        accum_out=s1[:],
                    )
                    x1s.append(x1)
                    lns.append(_ln_stats(nc, small, work, x1, s1, epsT))
                return x1s, lns

            def apply_transpose(qts, x1s, lns):
                for i, qt in enumerate(qts):
                    _ln_apply(nc, small, x1s[i], lns[i], z[:, qt, :])
                    for dt in range(ND):
                        pst = psA.tile([128, 128], F32, tag="a", name="pst")
                        nc.tensor.transpose(
                            pst[:], z[:, qt, dt * 128:(dt + 1) * 128], ident[:]
                        )
                        nc.scalar.copy(zT[:, dt, qt * 128:(qt + 1) * 128], pst[:])

            def ff1_chunk(qc):
                for et in range(ND):
                    ps = psA.tile([128, 512], F32, tag="a", name="ps_ff1")
                    for dt in range(ND):
                        nc.tensor.matmul(
                            ps[:],
                            W1gT[:, dt, et * 128:(et + 1) * 128],
                            zT[:, dt, qc * 512:(qc + 1) * 512],
                            start=(dt == 0), stop=(dt == ND - 1),
                        )
                    nc.scalar.activation(
                        ff1T[:, et, qc * 512:(qc + 1) * 512], ps[:],
                        AF.Relu, bias=c1[:, et:et + 1],
                    )

            st_chunk(0)
            a0, l0 = mha_stats([0, 1, 2, 3])
            st_chunk(1)
            apply_transpose([0, 1, 2, 3], a0, l0)
            a1, l1 = mha_stats([4, 5, 6, 7])
            ff1_chunk(0)

            # ---- phase 7: ff2 + residual + LN2 -> out (interleaved quarters) ----
            def ff2_stats(qts):
                rs, lns2 = [], []
                for qt in qts:
                    ps = psA.tile([128, 512], F32, tag="a", name="ps_ff2")
                    for et in range(ND):
                        nc.tensor.matmul(
                            ps[:],
                            ff1T[:, et, qt * 128:(qt + 1) * 128],
                            W2T[:, et, :],
                            start=(et == 0), stop=(fast and et == ND - 1),
                        )
                    if not fast:
                        nc.tensor.matmul(ps[:], onesr[:], C2[:], start=False, stop=True)
                    r = work.tile([128, D], F32, tag="r",
                                  bufs=NQ if fast else 4, name=f"r_{qt}")
                    s1b = small.tile([128, 1], F32, tag="s1")
                    if fast:
                        nc.vector.scalar_tensor_tensor(
                            r[:], ps[:], 1.0, z[:, qt, :],
                            op0=mybir.AluOpType.mult, op1=mybir.AluOpType.add,
                            accum_out=s1b[:],
                        )
                    else:
                        hres = work.tile([128, D], F32, tag="hres")
                        nc.vector.tensor_mul(hres[:], z[:, qt, :], g0b[:])
                        nc.vector.scalar_tensor_tensor(
                            r[:], ps[:], 1.0, hres[:],
                            op0=mybir.AluOpType.mult, op1=mybir.AluOpType.add,
                            accum_out=s1b[:],
                        )
                    rs.append(r)
                    lns2.append(_ln_stats(nc, small, work, r, s1b, epsT))
                return rs, lns2

            def ln2_out(qts, rs, lns2):
                for i, qt in enumerate(qts):
                    od = out_d.rearrange("(t p) n -> p t n", p=128)[:, qt, :]
                    z2 = work.tile([128, D], F32, tag="z2")
                    _ln_apply(nc, small, rs[i], lns2[i], z2[:])
                    if fast:
                        nc.sync.dma_start(od, z2[:])
                    else:
                        z2g = work.tile([128, D], F32, tag="sq")
                        nc.vector.tensor_mul(z2g[:], z2[:], g1b[:])
                        ot = work.tile([128, D], F32, tag="r2")
                        nc.vector.tensor_add(ot[:], z2g[:], be1b[:])
                        nc.sync.dma_start(od, ot[:])

            r01, l01 = ff2_stats([0, 1])
            apply_transpose([4, 5, 6, 7], a1, l1)
            ln2_out([0, 1], r01, l01)
            r23, l23 = ff2_stats([2, 3])
            ff1_chunk(1)
            ln2_out([2, 3], r23, l23)
            r45, l45 = ff2_stats([4, 5])
            ln2_out([4, 5], r45, l45)
            r67, l67 = ff2_stats([6, 7])
            ln2_out([6, 7], r67, l67)

    nc.compile()
    _BUILT[fast] = (nc,)
    return _BUILT[fast]


def _ln_stats(nc, small, work, x1, s1, epsT):
    """Stats for LN over the free axis: returns (nm, rstd) [128,1] tiles.
    s1 = row-sum of x1 (caller's accum_out); var = E[x^2] - mean^2."""
    sq = work.tile([128, D], F32, tag="sq")
    sqs = small.tile([128, 1], F32, tag="sqs")
    nc.scalar.activation(sq[:], x1[:], AF.Square, bias=0.0, accum_out=sqs[:])
    nm = small.tile([128, 1], F32, tag="nm")
    nc.scalar.mul(nm[:], s1[:], -1.0 / D)
    m2e = small.tile([128, 1], F32, tag="m2e")
    nc.vector.tensor_mul(m2e[:], nm[:], nm[:])
    nc.vector.tensor_scalar(m2e[:], m2e[:], -1.0, 1e-5,
                            op0=mybir.AluOpType.mult, op1=mybir.AluOpType.add)
    stdv = small.tile([128, 1], F32, tag="stdv")
    nc.scalar.activation(stdv[:], sqs[:], AF.Sqrt, scale=1.0 / D, bias=m2e[:])
    rstd = small.tile([128, 1], F32, tag="rstd")
    nc.vector.reciprocal(rstd[:], stdv[:])
    return nm, rstd


def _ln_apply(nc, small, x1, stats, out_ap):
    nm, rstd = stats
    # (x1 - mean) * rstd as one DVE op with two per-partition scalars
    nc.vector.tensor_scalar(out_ap, x1[:], nm[:], rstd[:],
                            op0=mybir.AluOpType.add, op1=mybir.AluOpType.mult)


def _prepare_in_maps(inputs):
    f64 = np.float64
    g = {k: np.asarray(v) for k, v in inputs.items()}
    x = g["x"].astype(f64)
    Wq, Wk, Wv = g["Wq"].astype(f64), g["Wk"].astype(f64), g["Wv"].astype(f64)
    Wo, W1, W2 = g["Wo"].astype(f64), g["W1"].astype(f64), g["W2"].astype(f64)
    bq, bk, bv, bo = g["bq"].astype(f64), g["bk"].astype(f64), g["bv"].astype(f64), g["bo"].astype(f64)
    b1, b2 = g["b1"].astype(f64), g["b2"].astype(f64)
    g0, be0, g1, be1 = g["g0"].astype(f64), g["be0"].astype(f64), g["g1"].astype(f64), g["be1"].astype(f64)

    fast = (
        not np.any(bq) and not np.any(bk) and not np.any(bv) and not np.any(bo)
        and not np.any(b1) and not np.any(b2) and not np.any(be0) and not np.any(be1)
        and bool(np.all(g0 == 1.0)) and bool(np.all(g1 == 1.0))
    )

    s = math.sqrt(D)
    f32 = lambda a: np.ascontiguousarray(a, dtype=np.float32)

    def wlay(W):  # [512, 512] -> SBUF layout [128, ND*512]
        return np.ascontiguousarray(
            W.reshape(ND, 128, D).transpose(1, 0, 2).reshape(128, ND * D),
            dtype=np.float32)

    Mw = wlay(Wq.T @ Wk / s)
    NT = wlay((Wo @ Wv).T)
    W1gT = wlay((W1 * g0[None, :]).T)
    W2T = wlay(W2.T)
    c1 = f32(b1 + W1 @ be0)
    wbo = Wo @ bv + bo
    vk = Wk.T @ bq / s

    shared = dict(Mw=Mw, NT=NT, W1gT=W1gT, W2T=W2T,
                  ident=np.eye(128, dtype=np.float32))
    if not fast:
        shared["C2"] = f32(b2 + be0).reshape(1, D)
        shared["onesr"] = np.ones((1, 128), np.float32)
        shared["g0b"] = f32(np.broadcast_to(g0, (128, D)))
        shared["g1b"] = f32(np.broadcast_to(g1, (128, D)))
        shared["be1b"] = f32(np.broadcast_to(be1, (128, D)))

    in_maps = []
    for b in range(B):
        xb = x[b]
        m = dict(shared)
        xTf = xb.T.reshape(ND, 128, 2, 512).transpose(2, 1, 0, 3)
        m["xT"] = f32(xTf.reshape(2, 128, ND * 512))
        m["xn"] = f32(xb.reshape(NQ, 128, D).transpose(1, 0, 2).reshape(128, NQ * D))
        smalls = np.zeros((128, 16), np.float32)
        smalls[:, 0:8] = f32(xb @ vk).reshape(8, 128).T
        smalls[:, 8:10] = 1.0
        smalls[:, 10:14] = c1.reshape(4, 128).T
        smalls[:, 14:15] = 1e-5
        m["smalls"] = smalls
        if not fast:
            m["xres"] = f32(xb + wbo[None, :])
        in_maps.append(m)
    return fast, in_maps


def _run(inputs, trace=False):
    fast, in_maps = _prepare_in_maps(inputs)
    (nc,) = _build(fast)
    res = run_bass_kernel_spmd(nc, in_maps, core_ids=list(range(B)), trace=trace)
    out = np.stack([res.results[c]["out"] for c in range(B)]).astype(np.float32)
    return out, res


def kernel(**inputs):
    out, _ = _run(inputs, trace=False)
    return out


# revision 30
# speedup vs baseline: 1.0062x; 1.0062x over previous
"""Trainium2 Bass kernel for nn_Encoder_55362128445616.

Transformer encoder layer: B=8, S=1024, D=512, single-head attention over
H*D=4096. Sharding: data-parallel over batch, one batch element per core,
no collectives.

Key algebraic folding (host-side, exact):
  scores = Q K^T / s = x (Wq^T Wk / s) x^T  (+ per-k bias from bq; per-q
  terms cancel in softmax), so the 4096-dim QKV contractions collapse to
  512-dim ones via M = Wq^T Wk / s and NT = (Wo Wv)^T.  LN1's affine is
  folded into W1/b1; all matmuls run as fp32r (1 cycle/row on PE).

Two compiled variants: `fast` (biases zero, LN affines identity — matches
the reference's setup_inputs) and a general fallback.
"""

import math

import numpy as np

# If the environment sets BASS_TRACE, bass_utils imports antenv.axon_hooks,
# which this image may lack — provide a no-op stub so plain runs never crash.
import sys as _sys
import types as _types
try:
    import antenv.axon_hooks  # noqa: F401
except ImportError:
    _m = _types.ModuleType("antenv.axon_hooks")
    _m.get_axon_ntff_profile_hook = lambda: None
    _m.set_axon_ntff_profile_hook = lambda hook: None
    _sys.modules["antenv.axon_hooks"] = _m

import concourse.bacc as bacc
import concourse.mybir as mybir
import concourse.tile as tile
from concourse.bass_utils import run_bass_kernel_spmd

B, S, D = 8, 1024, 512
NQ = S // 128   # 8 q/k tiles of 128
ND = D // 128   # 4 d tiles of 128
F32 = mybir.dt.float32
F32R = mybir.dt.float32r
AF = mybir.ActivationFunctionType
AX = mybir.AxisListType

_BUILT = {}


def _build(fast):
    if fast in _BUILT:
        return _BUILT[fast]

    nc = bacc.Bacc("TRN2", target_bir_lowering=False, debug=False, num_devices=B)

    def din(name, shape, dt=F32R):
        return nc.dram_tensor(name, shape, dt, kind="ExternalInput").ap()

    # all big inputs are pre-arranged on host to the exact SBUF layout so
    # every load is one fully-contiguous DMA at max HBM rate
    xT_d = din("xT", [2, 128, ND * 512])  # x[b].T as [q-half][p][bt*512]
    xn_d = din("xn", [128, NQ * D])       # x[b] as [p][kt*D]
    M_d = din("Mw", [128, ND * D])        # Wq^T Wk / sqrt(D), [p][bt*D]
    NT_d = din("NT", [128, ND * D])
    W1gT_d = din("W1gT", [128, ND * D])
    W2T_d = din("W2T", [128, ND * D])
    # packed per-partition columns: [0:8]=abias, [8:10]=ones, [10:14]=c1, [14:15]=eps
    sm_d = din("smalls", [128, 16], F32)
    id_d = din("ident", [128, 128], F32)
    if not fast:
        xres_d = din("xres", [S, D], F32)     # x[b] + (Wo@bv + bo)
        C2_d = din("C2", [1, D])              # b2 + be0
        onesr_d = din("onesr", [1, 128])
        g0b_d = din("g0b", [128, D], F32)
        g1b_d = din("g1b", [128, D], F32)
        be1b_d = din("be1b", [128, D], F32)
    out_d = nc.dram_tensor("out", [S, D], F32, kind="ExternalOutput").ap()

    with tile.TileContext(nc) as tc:
        with (
            tc.tile_pool(name="res", bufs=1) as res,
            tc.tile_pool(name="work", bufs=2) as work,
            tc.tile_pool(name="small", bufs=8) as small,
            tc.tile_pool(name="psA", bufs=3, space="PSUM") as psA,
            tc.tile_pool(name="psS", bufs=2, space="PSUM") as psS,
            tc.tile_pool(name="psD", bufs=1, space="PSUM") as psD,
        ):
            # ---- resident loads: sync (HWDGE) carries the phase-1/2 critical
            # path in use-order; gpsimd (SWDGE) carries later-phase tensors. ----
            Mw = res.tile([128, ND, D], F32R)
            nc.sync.dma_start(Mw[:], M_d.rearrange("p (t n) -> p t n", n=D))
            xT = res.tile([128, ND, S], F32R, tag="zx")
            for h in range(2):
                nc.sync.dma_start(
                    xT[:, :, h * 512:(h + 1) * 512],
                    xT_d[h].rearrange("p (t q) -> p t q", q=512))
            sm = res.tile([128, 16], F32)
            nc.sync.dma_start(sm[:], sm_d[:])
            xn = res.tile([128, NQ, D], F32R)
            nc.sync.dma_start(xn[:], xn_d.rearrange("p (t n) -> p t n", n=D))

            ident = res.tile([128, 128], F32)
            nc.sync.dma_start(ident[:], id_d[:])
            NTw = res.tile([128, ND, D], F32R)
            nc.sync.dma_start(NTw[:], NT_d.rearrange("p (t n) -> p t n", n=D))
            W1gT = res.tile([128, ND, D], F32R)
            nc.sync.dma_start(W1gT[:], W1gT_d.rearrange("p (t n) -> p t n", n=D))
            W2T = res.tile([128, ND, D], F32R)
            nc.sync.dma_start(W2T[:], W2T_d.rearrange("p (t n) -> p t n", n=D))
            if not fast:
                xres = res.tile([128, NQ, D], F32)
                nc.sync.dma_start(xres[:], xres_d.rearrange("(t p) n -> p t n", p=128))
                C2 = res.tile([1, D], F32R)
                nc.sync.dma_start(C2[:], C2_d[:])
                onesr = res.tile([1, 128], F32R)
                nc.sync.dma_start(onesr[:], onesr_d[:])
                g0b = res.tile([128, D], F32)
                nc.sync.dma_start(g0b[:], g0b_d[:])
                g1b = res.tile([128, D], F32)
                nc.sync.dma_start(g1b[:], g1b_d[:])
                be1b = res.tile([128, D], F32)
                nc.sync.dma_start(be1b[:], be1b_d[:])

            abias = sm[:, 0:8]
            onesc = sm[:, 8:10]
            c1 = sm[:, 10:14]
            epsT = sm[:, 14:15]

            # HAM warm-up on an uninitialized scratch tile: no input deps, so
            # the PE starts immediately while the first DMAs land.
            wtile = res.tile([128, 128], F32)
            nc.vector.memset(wtile[:], 0.0)
            for w in range(20):
                psw = psA.tile([128, 128], F32, tag="a", name=f"psw{w}")
                nc.tensor.matmul(psw[:], wtile[:], wtile[:], start=True, stop=True)

            # big SBUF intermediates; "mid"/"zx" tags reuse slots
            IN2 = res.tile([128, ND, S], F32R, tag="mid")   # M^T x^T
            PT = res.tile([128, NQ, S], F32R)               # exp(scores^T)
            zT = res.tile([128, ND, S], F32R)
            ff1T = res.tile([128, ND, S], F32R)

            # ---- phase 1: IN2[a, q] = sum_b M[b, a] xT[b, q] ----
            for qc in range(2):
                for at in range(ND):
                    ps = psA.tile([128, 512], F32, tag="a")
                    for bt in range(ND):
                        nc.tensor.matmul(
                            ps[:],
                            Mw[:, bt, at * 128:(at + 1) * 128],
                            xT[:, bt, qc * 512:(qc + 1) * 512],
                            start=(bt == 0), stop=(bt == ND - 1),
                        )
                    nc.vector.tensor_copy(IN2[:, at, qc * 512:(qc + 1) * 512], ps[:])

            # ---- phase 2: scoresT[k, q] = sum_a x[k, a] IN2[a, q]; PT = exp ----
            for kt in range(NQ):
                ps = psS.tile([128, 1024], F32, tag="s")
                for qc in range(2):
                    for at in range(ND):
                        nc.tensor.matmul(
                            ps[:, qc * 512:(qc + 1) * 512],
                            xT[:, at, kt * 128:(kt + 1) * 128],
                            IN2[:, at, qc * 512:(qc + 1) * 512],
                            start=(at == 0), stop=(at == ND - 1),
                        )
                bias = 0.0 if fast else abias[:, kt:kt + 1]
                nc.scalar.activation(PT[:, kt, :], ps[:], AF.Exp, bias=bias)

            # denomT[q] = sum_k PT[k, q]: DVE tree-sums the 8 k-tiles, then one
            # cheap f32 ones-matmul per q-tile flips [k-part, q] to [q-part, 1].
            dps = psD.tile([128, 2 * NQ], F32, tag="d")
            for qc in range(2):
                qs = slice(qc * 512, (qc + 1) * 512)
                dacc = work.tile([128, 512], F32, tag="sq", name=f"dacc{qc}")
                nc.vector.tensor_add(dacc[:], PT[:, 0, qs].bitcast(F32), PT[:, 1, qs].bitcast(F32))
                for kt in range(2, NQ):
                    nc.vector.tensor_add(dacc[:], dacc[:], PT[:, kt, qs].bitcast(F32))
                for ql in range(4):
                    qt = qc * 4 + ql
                    nc.tensor.matmul(
                        dps[:, 2 * qt:2 * qt + 2],
                        dacc[:, ql * 128:(ql + 1) * 128],
                        onesc[:],
                        start=True, stop=True,
                    )
            recip = res.tile([128, 2 * NQ], F32)
            nc.vector.reciprocal(recip[:], dps[:])

            # ---- phases 3-6, interleaved so PE work (ST qc1, ff1 chunks)
            # fills the LN1-chain latency windows ----
            ST = res.tile([128, ND, S], F32R, tag="mid")
            z = res.tile([128, NQ, D], F32, tag="zx")

            def st_chunk(qc):
                for dt in range(ND):
                    ps = psA.tile([128, 512], F32, tag="a", name="ps_st")
                    for kt in range(NQ):
                        nc.tensor.matmul(
                            ps[:],
                            xn[:, kt, dt * 128:(dt + 1) * 128],
                            PT[:, kt, qc * 512:(qc + 1) * 512],
                            start=(kt == 0), stop=(kt == NQ - 1),
                        )
                    nc.scalar.copy(ST[:, dt, qc * 512:(qc + 1) * 512], ps[:])

            def mha_stats(qts):
                x1s, lns = [], []
                for qt in qts:
                    ps = psA.tile([128, 512], F32, tag="a", name="ps_mha")
                    for dt in range(ND):
                        nc.tensor.matmul(
                            ps[:],
                            ST[:, dt, qt * 128:(qt + 1) * 128],
                            NTw[:, dt, :],
                            start=(dt == 0), stop=(dt == ND - 1),
                        )
                    x1 = work.tile([128, D], F32, tag="x1",
                                   bufs=NQ if fast else 4, name=f"x1_{qt}")
                    s1 = small.tile([128, 1], F32, tag="s1")
                    resid = xn[:, qt, :].bitcast(F32) if fast else xres[:, qt, :]
                    nc.vector.scalar_tensor_tensor(
                        x1[:], ps[:], recip[:, 2 * qt:2 * qt + 1], resid,
                        op0=mybir.AluOpType.mult, op1=mybir.AluOpType.add,
                        accum_out=s1[:],
                    )
                    x1s.append(x1)
                    lns.append(_ln_stats(nc, small, work, x1, s1, epsT))
                return x1s, lns

            def apply_transpose(qts, x1s, lns):
                for i, qt in enumerate(qts):
                    _ln_apply(nc, small, x1s[i], lns[i], z[:, qt, :])
                    for dt in range(ND):
                        pst = psA.tile([128, 128], F32, tag="a", name="pst")
                        nc.tensor.transpose(
                            pst[:], z[:, qt, dt * 128:(dt + 1) * 128], ident[:]
                        )
                        nc.scalar.copy(zT[:, dt, qt * 128:(qt + 1) * 128], pst[:])

            def ff1_chunk(qc):
                for et in range(ND):
                    ps = psA.tile([128, 512], F32, tag="a", name="ps_ff1")
                    for dt in range(ND):
                        nc.tensor.matmul(
                            ps[:],
                            W1gT[:, dt, et * 128:(et + 1) * 128],
                            zT[:, dt, qc * 512:(qc + 1) * 512],
                            start=(dt == 0), stop=(dt == ND - 1),
                        )
                    nc.scalar.activation(
                        ff1T[:, et, qc * 512:(qc + 1) * 512], ps[:],
                        AF.Relu, bias=c1[:, et:et + 1],
                    )

            st_chunk(0)
            a0, l0 = mha_stats([0, 1, 2, 3])
            st_chunk(1)
            apply_transpose([0, 1, 2, 3], a0, l0)
            a1, l1 = mha_stats([4, 5, 6, 7])
            ff1_chunk(0)

            # ---- phase 7: ff2 + residual + LN2 -> out (interleaved quarters) ----
            def ff2_stats(qts):
                rs, lns2 = [], []
                for qt in qts:
                    ps = psA.tile([128, 512], F32, tag="a", name="ps_ff2")
                    for et in range(ND):
                        nc.tensor.matmul(
                            ps[:],
                            ff1T[:, et, qt * 128:(qt + 1) * 128],
                            W2T[:, et, :],
                            start=(et == 0), stop=(fast and et == ND - 1),
                        )
                    if not fast:
                        nc.tensor.matmul(ps[:], onesr[:], C2[:], start=False, stop=True)
                    r = work.tile([128, D], F32, tag="r",
                                  bufs=NQ if fast else 4, name=f"r_{qt}")
                    s1b = small.tile([128, 1], F32, tag="s1")
                    if fast:
                        nc.vector.scalar_tensor_tensor(
                            r[:], ps[:], 1.0, z[:, qt, :],
                            op0=mybir.AluOpType.mult, op1=mybir.AluOpType.add,
                            accum_out=s1b[:],
                        )
                    else:
                        hres = work.tile([128, D], F32, tag="hres")
                        nc.vector.tensor_mul(hres[:], z[:, qt, :], g0b[:])
                        nc.vector.scalar_tensor_tensor(
                            r[:], ps[:], 1.0, hres[:],
                            op0=mybir.AluOpType.mult, op1=mybir.AluOpType.add,
                            accum_out=s1b[:],
                        )
                    rs.append(r)
                    lns2.append(_ln_stats(nc, small, work, r, s1b, epsT))
                return rs, lns2

            def ln2_out(qts, rs, lns2):
                for i, qt in enumerate(qts):
                    od = out_d.rearrange("(t p) n -> p t n", p=128)[:, qt, :]
                    z2 = work.tile([128, D], F32, tag="z2")
                    _ln_apply(nc, small, rs[i], lns2[i], z2[:])
                    if fast:
                        nc.sync.dma_start(od, z2[:])
                    else:
                        z2g = work.tile([128, D], F32, tag="sq")
                        nc.vector.tensor_mul(z2g[:], z2[:], g1b[:])
                        ot = work.tile([128, D], F32, tag="r2")
                        nc.vector.tensor_add(ot[:], z2g[:], be1b[:])
                        nc.sync.dma_start(od, ot[:])

            r01, l01 = ff2_stats([0, 1])
            apply_transpose([4, 5, 6, 7], a1, l1)
            ff1_chunk(1)
            ln2_out([0, 1], r01, l01)
            r23, l23 = ff2_stats([2, 3])
            ln2_out([2, 3], r23, l23)
            r45, l45 = ff2_stats([4, 5])
            r67, l67 = ff2_stats([6, 7])
            ln2_out([4, 5], r45, l45)
            ln2_out([6, 7], r67, l67)

    nc.compile()
    _BUILT[fast] = (nc,)
    return _BUILT[fast]


def _ln_stats(nc, small, work, x1, s1, epsT):
    """Stats for LN over the free axis: returns (nm, rstd) [128,1] tiles.
    s1 = row-sum of x1 (caller's accum_out); var = E[x^2] - mean^2."""
    sq = work.tile([128, D], F32, tag="sq")
    sqs = small.tile([128, 1], F32, tag="sqs")
    nc.scalar.activation(sq[:], x1[:], AF.Square, bias=0.0, accum_out=sqs[:])
    nm = small.tile([128, 1], F32, tag="nm")
    nc.scalar.mul(nm[:], s1[:], -1.0 / D)
    m2e = small.tile([128, 1], F32, tag="m2e")
    nc.vector.tensor_mul(m2e[:], nm[:], nm[:])
    nc.vector.tensor_scalar(m2e[:], m2e[:], -1.0, 1e-5,
                            op0=mybir.AluOpType.mult, op1=mybir.AluOpType.add)
    stdv = small.tile([128, 1], F32, tag="stdv")
    nc.scalar.activation(stdv[:], sqs[:], AF.Sqrt, scale=1.0 / D, bias=m2e[:])
    rstd = small.tile([128, 1], F32, tag="rstd")
    nc.vector.reciprocal(rstd[:], stdv[:])
    return nm, rstd


def _ln_apply(nc, small, x1, stats, out_ap):
    nm, rstd = stats
    # (x1 - mean) * rstd as one DVE op with two per-partition scalars
    nc.vector.tensor_scalar(out_ap, x1[:], nm[:], rstd[:],
                            op0=mybir.AluOpType.add, op1=mybir.AluOpType.mult)


def _prepare_in_maps(inputs):
    f64 = np.float64
    g = {k: np.asarray(v) for k, v in inputs.items()}
    x = g["x"].astype(f64)
    Wq, Wk, Wv = g["Wq"].astype(f64), g["Wk"].astype(f64), g["Wv"].astype(f64)
    Wo, W1, W2 = g["Wo"].astype(f64), g["W1"].astype(f64), g["W2"].astype(f64)
    bq, bk, bv, bo = g["bq"].astype(f64), g["bk"].astype(f64), g["bv"].astype(f64), g["bo"].astype(f64)
    b1, b2 = g["b1"].astype(f64), g["b2"].astype(f64)
    g0, be0, g1, be1 = g["g0"].astype(f64), g["be0"].astype(f64), g["g1"].astype(f64), g["be1"].astype(f64)

    fast = (
        not np.any(bq) and not np.any(bk) and not np.any(bv) and not np.any(bo)
        and not np.any(b1) and not np.any(b2) and not np.any(be0) and not np.any(be1)
        and bool(np.all(g0 == 1.0)) and bool(np.all(g1 == 1.0))
    )

    s = math.sqrt(D)
    f32 = lambda a: np.ascontiguousarray(a, dtype=np.float32)

    def wlay(W):  # [512, 512] -> SBUF layout [128, ND*512]
        return np.ascontiguousarray(
            W.reshape(ND, 128, D).transpose(1, 0, 2).reshape(128, ND * D),
            dtype=np.float32)

    Mw = wlay(Wq.T @ Wk / s)
    NT = wlay((Wo @ Wv).T)
    W1gT = wlay((W1 * g0[None, :]).T)
    W2T = wlay(W2.T)
    c1 = f32(b1 + W1 @ be0)
    wbo = Wo @ bv + bo
    vk = Wk.T @ bq / s

    shared = dict(Mw=Mw, NT=NT, W1gT=W1gT, W2T=W2T,
                  ident=np.eye(128, dtype=np.float32))
    if not fast:
        shared["C2"] = f32(b2 + be0).reshape(1, D)
        shared["onesr"] = np.ones((1, 128), np.float32)
        shared["g0b"] = f32(np.broadcast_to(g0, (128, D)))
        shared["g1b"] = f32(np.broadcast_to(g1, (128, D)))
        shared["be1b"] = f32(np.broadcast_to(be1, (128, D)))

    in_maps = []
    for b in range(B):
        xb = x[b]
        m = dict(shared)
        xTf = xb.T.reshape(ND, 128, 2, 512).transpose(2, 1, 0, 3)
        m["xT"] = f32(xTf.reshape(2, 128, ND * 512))
        m["xn"] = f32(xb.reshape(NQ, 128, D).transpose(1, 0, 2).reshape(128, NQ * D))
        smalls = np.zeros((128, 16), np.float32)
        smalls[:, 0:8] = f32(xb @ vk).reshape(8, 128).T
        smalls[:, 8:10] = 1.0
        smalls[:, 10:14] = c1.reshape(4, 128).T
        smalls[:, 14:15] = 1e-5
        m["smalls"] = smalls
        if not fast:
            m["xres"] = f32(xb + wbo[None, :])
        in_maps.append(m)
    return fast, in_maps


def _run(inputs, trace=False):
    fast, in_maps = _prepare_in_maps(inputs)
    (nc,) = _build(fast)
    res = run_bass_kernel_spmd(nc, in_maps, core_ids=list(range(B)), trace=trace)
    out = np.stack([res.results[c]["out"] for c in range(B)]).astype(np.float32)
    return out, res


def kernel(**inputs):
    out, _ = _run(inputs, trace=False)
    return out
